# revision 4
# baseline (speedup 1.0000x reference)
"""Cross-attention kernel for 8 TRN2 NeuronCores.

Strategy: pure data-parallel over batch B=64 -> 8 batches/core. Each core
runs the full per-batch pipeline; weights are replicated. All activations
are kept feature-major ("transposed": [features, tokens]) so every matmul
maps onto the PE array with zero on-chip layout transposes except the
structurally-required attention-probability transpose (done on the PE).

RoPE is applied as q_rope = cos*y + sin*(R @ y) where R is the fixed
64x64 pair-rotation matrix, computed with one extra PE matmul per tile
(blockdiag(R^T, R^T) stationary) instead of partition-strided shuffles.

Compute dtype: bf16 operands with fp32 PSUM accumulation; softmax in fp32.
"""

import numpy as np
import ml_dtypes
from contextlib import ExitStack

import concourse.bass as bass
import concourse.tile as tile
from concourse import bacc, mybir
from concourse.bass_utils import run_bass_kernel_spmd

# ---- problem constants (hardcoded per contract) ----
B, N, C, SEM = 64, 256, 1024, 768
H, HD = 16, 64
NCORES = 8
BPC = B // NCORES          # batches per core
T = BPC * N                # tokens per core (2048)
P = 128
KQ = C // P                # 8 contraction tiles for q-proj
KS = SEM // P              # 6 contraction tiles for kv-proj
M = C // P                 # 8 output-feature tiles
G = 4                      # token groups per core
GT = T // G                # tokens per group (512)
NB = 2                     # batches per group
PT_SEQ_LEN = 16
THETA = 10000.0

BF = mybir.dt.bfloat16
F32 = mybir.dt.float32
bf16 = ml_dtypes.bfloat16


def _rope_tables_np():
    d = HD // 2                                         # 32
    freqs = 1.0 / (THETA ** (np.arange(0, d, 2, dtype=np.float64) / d))   # (16,)
    t = np.arange(PT_SEQ_LEN, dtype=np.float64)
    f = np.einsum('i,j->ij', t, freqs)                  # (16, 16)
    f = np.repeat(f, 2, axis=-1)                        # (16, 32)
    fa = np.broadcast_to(f[:, None, :], (PT_SEQ_LEN, PT_SEQ_LEN, d))
    fb = np.broadcast_to(f[None, :, :], (PT_SEQ_LEN, PT_SEQ_LEN, d))
    full = np.concatenate([fa, fb], axis=-1).reshape(-1, HD)   # (256, 64)
    return np.cos(full).astype(np.float32), np.sin(full).astype(np.float32)


def _host_constants():
    cos, sin = _rope_tables_np()                        # (256, 64) each
    cosT = np.ascontiguousarray(cos.T)                  # (64, 256)
    sinT = np.ascontiguousarray(sin.T)
    cosrep = np.tile(cosT, (2, 2))                      # (128, 512)
    sinrep = np.tile(sinT, (2, 2))
    scale = 1.0 / np.sqrt(np.float32(HD))               # 0.125, folded into q side
    consts = {
        "cosq": (cosrep * scale).astype(bf16),
        "sinq": (sinrep * scale).astype(bf16),
        "cosk": cosrep.astype(bf16),
        "sink": sinrep.astype(bf16),
    }
    # RT2 = blockdiag(R^T, R^T): psum = RT2.T @ y = rot(y)
    RT = np.zeros((HD, HD), np.float32)
    for i in range(HD // 2):
        RT[2 * i + 1, 2 * i] = -1.0
        RT[2 * i, 2 * i + 1] = 1.0
    RT2 = np.zeros((P, P), np.float32)
    RT2[:HD, :HD] = RT
    RT2[HD:, HD:] = RT
    consts["RT2"] = RT2.astype(bf16)
    consts["ident"] = np.eye(P, dtype=bf16)
    return consts


def _body(ctx: ExitStack, tc: "tile.TileContext", io: dict):
    nc = tc.nc

    wpool = ctx.enter_context(tc.tile_pool(name="weights", bufs=1))
    const = ctx.enter_context(tc.tile_pool(name="const", bufs=1))
    inq = ctx.enter_context(tc.tile_pool(name="inq", bufs=2))
    inkv = ctx.enter_context(tc.tile_pool(name="inkv", bufs=2))
    acts = ctx.enter_context(tc.tile_pool(name="acts", bufs=2))
    tmp = ctx.enter_context(tc.tile_pool(name="tmp", bufs=3))
    attnp = ctx.enter_context(tc.tile_pool(name="attnp", bufs=3))
    aop = ctx.enter_context(tc.tile_pool(name="aop", bufs=2))
    outp = ctx.enter_context(tc.tile_pool(name="outp", bufs=3))
    small = ctx.enter_context(tc.tile_pool(name="small", bufs=6))
    ps = ctx.enter_context(tc.tile_pool(name="ps", bufs=5, space="PSUM"))
    ps_t = ctx.enter_context(tc.tile_pool(name="ps_t", bufs=3, space="PSUM"))

    # ---- resident weights & constants ----
    Wq_sb = []
    for k in range(KQ):
        t_ = wpool.tile([P, C], BF, tag=f"wq{k}")
        nc.sync.dma_start(t_[:], io["Wq"][k * P:(k + 1) * P, :])
        Wq_sb.append(t_)
    Wk_sb, Wv_sb = [], []
    for k in range(KS):
        t_ = wpool.tile([P, C], BF, tag=f"wk{k}")
        nc.sync.dma_start(t_[:], io["Wk"][k * P:(k + 1) * P, :])
        Wk_sb.append(t_)
        t_ = wpool.tile([P, C], BF, tag=f"wv{k}")
        nc.sync.dma_start(t_[:], io["Wv"][k * P:(k + 1) * P, :])
        Wv_sb.append(t_)
    Wp_sb = []
    for k in range(M):
        t_ = wpool.tile([P, C], BF, tag=f"wp{k}")
        nc.sync.dma_start(t_[:], io["Wproj"][k * P:(k + 1) * P, :])
        Wp_sb.append(t_)

    cn = {}
    for name, shape in [("cosq", [P, GT]), ("sinq", [P, GT]),
                        ("cosk", [P, GT]), ("sink", [P, GT]),
                        ("RT2", [P, P]), ("ident", [P, P])]:
        t_ = const.tile(shape, BF, tag=name)
        nc.sync.dma_start(t_[:], io[name][:])
        cn[name] = t_
    bprojT = const.tile([P, M], F32, tag="bprojT")
    nc.sync.dma_start(bprojT[:], io["bprojT"][:])

    for g in range(G):
        c0 = g * GT
        # ---- group input staging ----
        qTg = []
        for k in range(KQ):
            t_ = inq.tile([P, GT], BF, tag=f"qTg{k}")
            nc.sync.dma_start(t_[:], io["qT"][k * P:(k + 1) * P, c0:c0 + GT])
            qTg.append(t_)
        kvTg = []
        for k in range(KS):
            t_ = inkv.tile([P, GT], BF, tag=f"kvTg{k}")
            nc.sync.dma_start(t_[:], io["kvT"][k * P:(k + 1) * P, c0:c0 + GT])
            kvTg.append(t_)

        qrope = acts.tile([P, M, GT], BF, tag="qrope")
        krope = acts.tile([P, M, GT], BF, tag="krope")
        Vt = acts.tile([P, G, C], BF, tag="Vt")   # [tok128, tok-tile, vfeat]

        # ---- Q / K projections + RoPE ----
        for dst, Wsb, src, nk, cosA, sinA in (
            (qrope, Wq_sb, qTg, KQ, cn["cosq"], cn["sinq"]),
            (krope, Wk_sb, kvTg, KS, cn["cosk"], cn["sink"]),
        ):
            for m in range(M):
                acc = ps.tile([P, GT], F32, tag="ps")
                for k in range(nk):
                    nc.tensor.matmul(acc[:], Wsb[k][:, m * P:(m + 1) * P], src[k][:],
                                     start=(k == 0), stop=(k == nk - 1))
                y = tmp.tile([P, GT], BF, tag="y")
                nc.scalar.copy(y[:], acc[:])
                rot = ps.tile([P, GT], F32, tag="ps")
                nc.tensor.matmul(rot[:], cn["RT2"][:], y[:], start=True, stop=True)
                r = tmp.tile([P, GT], BF, tag="r")
                nc.scalar.copy(r[:], rot[:])
                t1 = tmp.tile([P, GT], BF, tag="t1")
                nc.vector.tensor_mul(t1[:], y[:], cosA[:])
                t2 = tmp.tile([P, GT], BF, tag="t2")
                nc.vector.tensor_mul(t2[:], r[:], sinA[:])
                nc.vector.tensor_add(dst[:, m, :], t1[:], t2[:])

        # ---- V projection (token-major via kvT as stationary) ----
        for tt in range(4):
            for nn in range(2):
                acc = ps.tile([P, GT], F32, tag="ps")
                for k in range(KS):
                    nc.tensor.matmul(
                        acc[:], kvTg[k][:, tt * P:(tt + 1) * P],
                        Wv_sb[k][:, nn * GT:(nn + 1) * GT],
                        start=(k == 0), stop=(k == KS - 1))
                nc.scalar.copy(Vt[:, tt, nn * GT:(nn + 1) * GT], acc[:])

        # ---- attention + output projection, per batch in group ----
        for bb in range(NB):
            t0 = bb * N
            ao = aop.tile([P, M, N], BF, tag="ao")
            for hp in range(M):
                pso = ps.tile([P, N], F32, tag="ps")
                for sub in range(2):
                    h = 2 * hp + sub
                    p0 = sub * HD
                    Pn = attnp.tile([P, 2, N], BF, tag=f"Pn{sub}")
                    for qt in range(2):
                        psl = ps.tile([P, N], F32, tag="ps")
                        nc.tensor.matmul(
                            psl[:],
                            qrope[p0:p0 + HD, hp, t0 + qt * P: t0 + (qt + 1) * P],
                            krope[p0:p0 + HD, hp, t0:t0 + N],
                            start=True, stop=True)
                        nm = small.tile([P, 1], F32, tag="nm")
                        nc.vector.tensor_reduce(
                            nm[:], psl[:], axis=mybir.AxisListType.X,
                            op=mybir.AluOpType.max, negate=True)
                        ssum = small.tile([P, 1], F32, tag="ssum")
                        nc.scalar.activation(
                            Pn[:, qt, :], psl[:],
                            mybir.ActivationFunctionType.Exp,
                            bias=nm[:], scale=1.0, accum_out=ssum[:])
                        rinv = small.tile([P, 1], F32, tag="rinv")
                        nc.vector.reciprocal(rinv[:], ssum[:])
                        nc.vector.tensor_scalar_mul(Pn[:, qt, :], Pn[:, qt, :], rinv[:])
                    # transpose P -> P^T per k-slice
                    PTs = []
                    for kt in range(2):
                        pt_sb = attnp.tile([P, N], BF, tag=f"PT{sub}{kt}")
                        PTs.append(pt_sb)
                        for qt in range(2):
                            pst = ps_t.tile([P, P], BF, tag="pst")
                            nc.tensor.transpose(
                                pst[:], Pn[:, qt, kt * P:(kt + 1) * P], cn["ident"][:])
                            nc.scalar.copy(pt_sb[:, qt * P:(qt + 1) * P], pst[:])
                    # attn @ v  ->  out^T block (col-packed by head parity)
                    for kt in range(2):
                        nc.tensor.matmul(
                            pso[p0:p0 + HD, :],
                            Vt[:, bb * 2 + kt, h * HD:(h + 1) * HD],
                            PTs[kt][:],
                            start=(kt == 0), stop=(kt == 1),
                            tile_position=(0, p0))
                nc.scalar.copy(ao[:, hp, :], pso[:])
            # output projection for this batch
            for m in range(M):
                psf = ps.tile([P, N], F32, tag="ps")
                for k2 in range(M):
                    nc.tensor.matmul(psf[:], Wp_sb[k2][:, m * P:(m + 1) * P],
                                     ao[:, k2, :], start=(k2 == 0), stop=(k2 == M - 1))
                osb = outp.tile([P, N], F32, tag="osb")
                nc.scalar.add(osb[:], psf[:], add=bprojT[:, m:m + 1])
                nc.sync.dma_start(
                    io["outT"][m * P:(m + 1) * P, c0 + t0:c0 + t0 + N], osb[:])


_CACHED_NC = None


def _build_nc():
    global _CACHED_NC
    if _CACHED_NC is not None:
        return _CACHED_NC
    nc = bacc.Bacc("TRN2", target_bir_lowering=False, debug=False,
                   num_devices=NCORES)
    io = {}
    def din(name, shape, dt=BF):
        io[name] = nc.dram_tensor(name, shape, dt, kind="ExternalInput").ap()
    din("qT", [C, T])
    din("kvT", [SEM, T])
    din("Wq", [C, C])
    din("Wk", [SEM, C])
    din("Wv", [SEM, C])
    din("Wproj", [C, C])
    din("cosq", [P, GT]); din("sinq", [P, GT])
    din("cosk", [P, GT]); din("sink", [P, GT])
    din("RT2", [P, P]); din("ident", [P, P])
    din("bprojT", [P, M], F32)
    io["outT"] = nc.dram_tensor("outT", [C, T], F32, kind="ExternalOutput").ap()

    with tile.TileContext(nc) as tc:
        with ExitStack() as ctx:
            _body(ctx, tc, io)
    nc.compile()
    _CACHED_NC = nc
    return nc


def kernel(q, kv, Wq, Wkv, Wproj, bproj, _trace=False, _trace_kwargs=None):
    nc = _build_nc()
    consts = _host_constants()
    shared = {
        "Wq": np.ascontiguousarray(Wq.astype(bf16)),
        "Wk": np.ascontiguousarray(Wkv[:, :C].astype(bf16)),
        "Wv": np.ascontiguousarray(Wkv[:, C:].astype(bf16)),
        "Wproj": np.ascontiguousarray(Wproj.astype(bf16)),
        "bprojT": np.ascontiguousarray(
            bproj.astype(np.float32).reshape(M, P).T),
        **consts,
    }
    in_maps = []
    for i in range(NCORES):
        qs = q[i * BPC:(i + 1) * BPC].reshape(T, C)
        kvs = kv[i * BPC:(i + 1) * BPC].reshape(T, SEM)
        in_maps.append({
            "qT": np.ascontiguousarray(qs.T.astype(bf16)),
            "kvT": np.ascontiguousarray(kvs.T.astype(bf16)),
            **shared,
        })
    kw = {}
    if _trace:
        kw.update(trace=True, **(_trace_kwargs or {}))
    res = run_bass_kernel_spmd(nc, in_maps, core_ids=list(range(NCORES)), **kw)
    out = np.empty((B, N, C), np.float32)
    for i in range(NCORES):
        out[i * BPC:(i + 1) * BPC] = (
            res.results[i]["outT"].T.reshape(BPC, N, C))
    if _trace:
        return out, res
    return out
